# revision 24
# baseline (speedup 1.0000x reference)
"""BatchBlur_SV kernel for 8 Trainium2 NeuronCores (Bass/Tile).

Reference semantics (including its reshape-scrambling "bug"):
  X = ker.reshape(361, 65536)                  # (kernel-pos ab, pixel p)
  s1 = X.sum(0);  W  = X / s1                  # stage-1 per-pixel normalize
  A2 = W.flat chunks of 361; s2 = row sums;  B2 = A2 / s2     # stage 2
  A3 = (B2.T).flat chunks of 361; s3 = row sums               # stage 3
  U  = im2col(reflect_pad(input[0,2], 9)) in (ab, p) layout   # (361, 65536)
  out[r] = sum(U.flat_chunk_r * A3[r]) / s3[r]

All arithmetic runs on-device in 3 SPMD launches over 8 cores, each core
working on a 1/8 flat band. Host only slices / rolls / transposes / casts
between launches (data movement, no math).

Perf notes vs the fp32 baseline (242us):
 - all large streamed tensors are bf16 (halves DMA bytes, doubles DVE rate);
   every reduction accumulates into fp32 tiles.
 - k1 emits 1/s1 (one small reciprocal) so k2 skips its full-size
   reciprocal pass and multiplies directly.
"""

import numpy as np
import ml_dtypes

BF16 = ml_dtypes.bfloat16

P = 65536          # pixels
L = 19
L2 = 361           # kernel positions
NCORES = 8
PS = P // NCORES   # 8192 rows per core
NB = PS * L2       # flat elements per band
G = 8              # subtiles per DMA group
NGRP = PS // (128 * G)   # 8 groups per core

_CACHE: dict = {}


def _dt():
    from concourse import mybir
    return mybir.dt.float32, mybir.dt.bfloat16


def _grouped(ap):
    # (PS, L2) -> [g][k][(i j)] with row = g*1024 + k*G + i: each partition
    # holds G consecutive rows, so src/dst DMA patterns are contiguous 2D.
    return ap.rearrange("(g k i) j -> g k (i j)", g=NGRP, k=128, i=G)


def _build_k1():
    """colsum+recip kernel: in xT (PS, 361) bf16 slice of X.T
    -> out rs1 (PS,) fp32 with rs1 = 1/colsum."""
    import concourse.bacc as bacc
    import concourse.tile as tile
    from concourse import mybir

    import bass_rust

    f32, bf16 = _dt()
    act_id = bass_rust.ActivationFunctionType.Identity
    nc = bacc.Bacc("TRN2", target_bir_lowering=False)
    xT = nc.dram_tensor("xT", [PS, L2], bf16, kind="ExternalInput")
    rs1 = nc.dram_tensor("rs1", [128, NGRP * G], f32, kind="ExternalOutput")
    xr = _grouped(xT[:, :])
    with tile.TileContext(nc) as tc:
        with (
            tc.tile_pool(name="io", bufs=3) as pool,
            tc.tile_pool(name="scr", bufs=2) as scrp,
            tc.tile_pool(name="acc", bufs=1) as accp,
        ):
            acc = accp.tile([128, NGRP, G], f32)
            racc = accp.tile([128, NGRP, G], f32)
            for g in range(NGRP):
                xt = pool.tile([128, G, L2], bf16)
                nc.sync.dma_start(
                    out=xt[:, :, :].rearrange("k i j -> k (i j)"), in_=xr[g]
                )
                if g % 8 < 5:
                    # DVE: one big row-sum for this group
                    nc.vector.tensor_reduce(
                        out=acc[:, g, :], in_=xt,
                        axis=mybir.AxisListType.X, op=mybir.AluOpType.add,
                    )
                else:
                    # Activation: row-sums via accumulate output
                    scr = scrp.tile([128, G, L2], bf16)
                    for i in range(G):
                        nc.scalar.activation(
                            out=scr[:, i, :], in_=xt[:, i, :], func=act_id,
                            accum_out=acc[:, g, i : i + 1],
                        )
            nc.vector.reciprocal(
                out=racc[:, :, :].rearrange("k g i -> k (g i)"),
                in_=acc[:, :, :].rearrange("k g i -> k (g i)"),
            )
            nc.sync.dma_start(out=rs1[:, :], in_=racc)
    nc.compile()
    return nc


def _build_k2():
    """stage-2 kernel: in a2 (PS,361) bf16 = X.flat band, rs1b (PS,361) bf16 =
    matching per-element 1/s1; out b2 (PS,361) bf16 normalized chunks."""
    import concourse.bacc as bacc
    import concourse.tile as tile
    from concourse import mybir

    f32, bf16 = _dt()
    nc = bacc.Bacc("TRN2", target_bir_lowering=False)
    a2 = nc.dram_tensor("a2", [PS, L2], bf16, kind="ExternalInput")
    s1b = nc.dram_tensor("s1b", [PS, L2], bf16, kind="ExternalInput")
    b2 = nc.dram_tensor("b2", [PS, L2], bf16, kind="ExternalOutput")
    a2r, s1r, b2r = _grouped(a2[:, :]), _grouped(s1b[:, :]), _grouped(b2[:, :])
    with tile.TileContext(nc) as tc:
        with (
            tc.tile_pool(name="io", bufs=3) as pool,
            tc.tile_pool(name="w", bufs=3) as wpool,
            tc.tile_pool(name="st", bufs=3) as spool,
        ):
            for g in range(NGRP):
                ta = pool.tile([128, G, L2], bf16, tag="ta")
                ts = pool.tile([128, G, L2], bf16, tag="ts")
                nc.sync.dma_start(
                    out=ta[:, :, :].rearrange("k i j -> k (i j)"), in_=a2r[g]
                )
                nc.sync.dma_start(
                    out=ts[:, :, :].rearrange("k i j -> k (i j)"), in_=s1r[g]
                )
                tw = wpool.tile([128, G, L2], bf16)
                nc.vector.tensor_mul(out=tw, in0=ta, in1=ts)
                s2 = spool.tile([128, G], f32, tag="s2")
                nc.vector.tensor_reduce(
                    out=s2, in_=tw,
                    axis=mybir.AxisListType.X, op=mybir.AluOpType.add,
                )
                r2 = spool.tile([128, G], f32, tag="r2")
                nc.vector.reciprocal(out=r2, in_=s2)
                for i in range(G):
                    # per-row scale on the Activation engine (frees DVE)
                    nc.scalar.mul(
                        out=tw[:, i, :], in_=tw[:, i, :],
                        mul=r2[:, i : i + 1],
                    )
                nc.sync.dma_start(
                    out=b2r[g], in_=tw[:, :, :].rearrange("k i j -> k (i j)")
                )
    nc.compile()
    return nc


def _build_k3():
    """final kernel: in v (PS,361) bf16 = B2T.flat band, u (PS,361) bf16 =
    U.flat band; out o (PS,) fp32 = rowdot(u,v)/rowsum(v)."""
    import concourse.bacc as bacc
    import concourse.tile as tile
    from concourse import mybir

    import bass_rust

    f32, bf16 = _dt()
    act_id = bass_rust.ActivationFunctionType.Identity
    nc = bacc.Bacc("TRN2", target_bir_lowering=False)
    v = nc.dram_tensor("v", [PS, L2], bf16, kind="ExternalInput")
    u = nc.dram_tensor("u", [PS, L2], bf16, kind="ExternalInput")
    o = nc.dram_tensor("o", [128, NGRP * G], f32, kind="ExternalOutput")
    vr, ur = _grouped(v[:, :]), _grouped(u[:, :])
    with tile.TileContext(nc) as tc:
        with (
            tc.tile_pool(name="io", bufs=3) as pool,
            tc.tile_pool(name="pr", bufs=2) as prp,
            tc.tile_pool(name="st", bufs=3) as spool,
            tc.tile_pool(name="acc", bufs=1) as accp,
        ):
            oacc = accp.tile([128, NGRP, G], f32)
            for g in range(NGRP):
                tv = pool.tile([128, G, L2], bf16, tag="tv")
                tu = pool.tile([128, G, L2], bf16, tag="tu")
                nc.sync.dma_start(
                    out=tv[:, :, :].rearrange("k i j -> k (i j)"), in_=vr[g]
                )
                nc.sync.dma_start(
                    out=tu[:, :, :].rearrange("k i j -> k (i j)"), in_=ur[g]
                )
                s3 = spool.tile([128, G], f32, tag="s3")
                scr = prp.tile([128, G, L2], bf16, tag="scr")
                for i in range(G):
                    # Activation: row-sums of v via accumulate output
                    nc.scalar.activation(
                        out=scr[:, i, :], in_=tv[:, i, :], func=act_id,
                        accum_out=s3[:, i : i + 1],
                    )
                prod = prp.tile([128, G, L2], bf16)
                nc.vector.tensor_mul(out=prod, in0=tu, in1=tv)
                dots = spool.tile([128, G], f32, tag="dots")
                nc.vector.tensor_reduce(
                    out=dots, in_=prod,
                    axis=mybir.AxisListType.X, op=mybir.AluOpType.add,
                )
                r3 = spool.tile([128, G], f32, tag="r3")
                nc.vector.reciprocal(out=r3, in_=s3)
                nc.vector.tensor_mul(out=oacc[:, g, :], in0=dots, in1=r3)
            nc.sync.dma_start(out=o[:, :], in_=oacc)
    nc.compile()
    return nc


def _run(key, builder, in_maps, trace=False):
    from concourse.bass_utils import run_bass_kernel_spmd

    if key not in _CACHE:
        _CACHE[key] = builder()
    res = run_bass_kernel_spmd(
        _CACHE[key], in_maps, core_ids=list(range(NCORES)), trace=trace
    )
    return res


def kernel(input, kernel):
    import os

    trace = bool(int(os.environ.get("BASSBLUR_TRACE", "0")))
    inp = np.ascontiguousarray(np.asarray(input, dtype=np.float32))
    ker = np.ascontiguousarray(np.asarray(kernel, dtype=np.float32))
    X = ker.reshape(L2, P)
    Xbf = X.astype(BF16)
    Xbff = Xbf.reshape(-1)

    times = []

    # ---- launch 1: rs1 = 1 / column sums of X -------------------------
    XTbf = np.ascontiguousarray(Xbf.T)  # (P, 361) bf16
    in1 = [
        {"xT": XTbf[m * PS : (m + 1) * PS]}
        for m in range(NCORES)
    ]
    r1 = _run("k1", _build_k1, in1, trace=trace)
    rs1 = np.concatenate(
        [r["rs1"].reshape(128, NGRP, G).transpose(1, 0, 2).ravel()
         for r in r1.results]
    )
    times.append(r1.exec_time_ns)

    # ---- launch 2: per-chunk stage-2 normalize ------------------------
    # band m covers flat [NB*m, NB*(m+1)); element x there needs
    # rs1[(NB*m + x) % P]; NB % P == PS so the roll shift is PS*m.
    in2 = []
    for m in range(NCORES):
        s1b = np.resize(np.roll(rs1, -(PS * m) % P), NB).reshape(PS, L2)
        in2.append(
            {
                "a2": Xbff[NB * m : NB * (m + 1)].reshape(PS, L2),
                "s1b": s1b.astype(BF16),
            }
        )
    r2 = _run("k2", _build_k2, in2, trace=trace)
    B2 = np.concatenate([r["b2"] for r in r2.results], axis=0)  # (P, 361) bf16
    times.append(r2.exec_time_ns)

    # ---- launch 3: final dot over B2T/U flat chunks -------------------
    B2Tf = np.ascontiguousarray(B2.T).reshape(-1)
    pad = np.pad(inp[0, 2], L // 2, mode="reflect")  # (274, 274)
    from numpy.lib.stride_tricks import sliding_window_view

    U = np.ascontiguousarray(
        sliding_window_view(pad, (256, 256)).reshape(L2, P)
    )
    Ubff = U.astype(BF16).reshape(-1)
    in3 = [
        {
            "v": B2Tf[NB * m : NB * (m + 1)].reshape(PS, L2),
            "u": Ubff[NB * m : NB * (m + 1)].reshape(PS, L2),
        }
        for m in range(NCORES)
    ]
    r3 = _run("k3", _build_k3, in3, trace=trace)
    out = np.concatenate(
        [r["o"].reshape(128, NGRP, G).transpose(1, 0, 2).ravel()
         for r in r3.results]
    )
    times.append(r3.exec_time_ns)

    if trace:
        kernel._last_times_ns = times  # stash for test harness

    return out.reshape(1, 1, 256, 256).astype(np.float32)


def hw_time_estimate_ns():
    """Per-launch HW time from the instruction cost model (TimelineSim).

    NTFF/neuron-profile capture is unavailable under this axon build, so this
    is the principled substitute: the same InstructionCostModel the Tile
    scheduler uses, over the exact BIR that runs on the cores.
    """
    from concourse.timeline_sim import TimelineSim

    out = []
    for key, builder in [("k1", _build_k1), ("k2", _build_k2), ("k3", _build_k3)]:
        if key not in _CACHE:
            _CACHE[key] = builder()
        out.append(int(TimelineSim(_CACHE[key]).simulate()))
    return out


# revision 27
# speedup vs baseline: 1.0676x; 1.0676x over previous
"""BatchBlur_SV kernel for 8 Trainium2 NeuronCores (Bass/Tile).

Reference semantics (including its reshape-scrambling "bug"):
  X = ker.reshape(361, 65536)                  # (kernel-pos ab, pixel p)
  s1 = X.sum(0);  W  = X / s1                  # stage-1 per-pixel normalize
  A2 = W.flat chunks of 361; s2 = row sums;  B2 = A2 / s2     # stage 2
  A3 = (B2.T).flat chunks of 361; s3 = row sums               # stage 3
  U  = im2col(reflect_pad(input[0,2], 9)) in (ab, p) layout   # (361, 65536)
  out[r] = sum(U.flat_chunk_r * A3[r]) / s3[r]

All arithmetic runs on-device in 3 SPMD launches over 8 cores, each core
working on a 1/8 flat band. Host only slices / rolls / transposes / casts
between launches (data movement, no math).

Perf notes vs the fp32 baseline (242us):
 - all large streamed tensors are bf16 (halves DMA bytes, doubles DVE rate);
   every reduction accumulates into fp32 tiles.
 - k1 emits 1/s1 (one small reciprocal) so k2 skips its full-size
   reciprocal pass and multiplies directly.
"""

import numpy as np
import ml_dtypes

BF16 = ml_dtypes.bfloat16

P = 65536          # pixels
L = 19
L2 = 361           # kernel positions
NCORES = 8
PS = P // NCORES   # 8192 rows per core
NB = PS * L2       # flat elements per band
G = 8              # subtiles per DMA group
NGRP = PS // (128 * G)   # 8 groups per core

_CACHE: dict = {}


def _dt():
    from concourse import mybir
    return mybir.dt.float32, mybir.dt.bfloat16


def _grouped(ap):
    # (PS, L2) -> [g][k][(i j)] with row = g*1024 + k*G + i: each partition
    # holds G consecutive rows, so src/dst DMA patterns are contiguous 2D.
    return ap.rearrange("(g k i) j -> g k (i j)", g=NGRP, k=128, i=G)


def _build_k1():
    """colsum+recip kernel: in xT (PS, 361) bf16 slice of X.T
    -> out rs1 (PS,) fp32 with rs1 = 1/colsum."""
    import concourse.bacc as bacc
    import concourse.tile as tile
    from concourse import mybir

    import bass_rust

    f32, bf16 = _dt()
    act_id = bass_rust.ActivationFunctionType.Identity
    nc = bacc.Bacc("TRN2", target_bir_lowering=False)
    xT = nc.dram_tensor("xT", [PS, L2], bf16, kind="ExternalInput")
    rs1 = nc.dram_tensor("rs1", [128, NGRP * G], f32, kind="ExternalOutput")
    xr = _grouped(xT[:, :])
    with tile.TileContext(nc) as tc:
        with (
            tc.tile_pool(name="io", bufs=3) as pool,
            tc.tile_pool(name="scr", bufs=2) as scrp,
            tc.tile_pool(name="acc", bufs=1) as accp,
        ):
            acc = accp.tile([128, NGRP, G], f32)
            racc = accp.tile([128, NGRP, G], f32)
            for g in range(NGRP):
                xt = pool.tile([128, G, L2], bf16)
                nc.sync.dma_start(
                    out=xt[:, :, :].rearrange("k i j -> k (i j)"), in_=xr[g]
                )
                nc.vector.tensor_reduce(
                    out=acc[:, g, :], in_=xt,
                    axis=mybir.AxisListType.X, op=mybir.AluOpType.add,
                )
            nc.vector.reciprocal(
                out=racc[:, :, :].rearrange("k g i -> k (g i)"),
                in_=acc[:, :, :].rearrange("k g i -> k (g i)"),
            )
            nc.sync.dma_start(out=rs1[:, :], in_=racc)
    nc.compile()
    return nc


def _build_k2():
    """stage-2 kernel: in a2 (PS,361) bf16 = X.flat band, rs1b (PS,361) bf16 =
    matching per-element 1/s1; out b2 (PS,361) bf16 normalized chunks."""
    import concourse.bacc as bacc
    import concourse.tile as tile
    from concourse import mybir

    import bass_rust

    f32, bf16 = _dt()
    act_id = bass_rust.ActivationFunctionType.Identity
    nc = bacc.Bacc("TRN2", target_bir_lowering=False)
    a2 = nc.dram_tensor("a2", [PS, L2], bf16, kind="ExternalInput")
    s1b = nc.dram_tensor("s1b", [PS, L2], bf16, kind="ExternalInput")
    b2 = nc.dram_tensor("b2", [PS, L2], bf16, kind="ExternalOutput")
    a2r, s1r, b2r = _grouped(a2[:, :]), _grouped(s1b[:, :]), _grouped(b2[:, :])
    with tile.TileContext(nc) as tc:
        with (
            tc.tile_pool(name="io", bufs=3) as pool,
            tc.tile_pool(name="w", bufs=3) as wpool,
            tc.tile_pool(name="st", bufs=3) as spool,
        ):
            for g in range(NGRP):
                ta = pool.tile([128, G, L2], bf16, tag="ta")
                ts = pool.tile([128, G, L2], bf16, tag="ts")
                nc.sync.dma_start(
                    out=ta[:, :, :].rearrange("k i j -> k (i j)"), in_=a2r[g]
                )
                nc.sync.dma_start(
                    out=ts[:, :, :].rearrange("k i j -> k (i j)"), in_=s1r[g]
                )
                tw = wpool.tile([128, G, L2], bf16)
                nc.vector.tensor_mul(out=tw, in0=ta, in1=ts)
                s2 = spool.tile([128, G], f32, tag="s2")
                scr = wpool.tile([128, G, L2], bf16, tag="scr")
                for i in range(G):
                    # Activation: row-sums of tw via accumulate output
                    nc.scalar.activation(
                        out=scr[:, i, :], in_=tw[:, i, :], func=act_id,
                        accum_out=s2[:, i : i + 1],
                    )
                r2 = spool.tile([128, G], f32, tag="r2")
                nc.vector.reciprocal(out=r2, in_=s2)
                for i in range(G):
                    nc.vector.tensor_scalar_mul(
                        out=tw[:, i, :], in0=tw[:, i, :],
                        scalar1=r2[:, i : i + 1],
                    )
                nc.sync.dma_start(
                    out=b2r[g], in_=tw[:, :, :].rearrange("k i j -> k (i j)")
                )
    nc.compile()
    return nc


def _build_k3():
    """final kernel: in v (PS,361) bf16 = B2T.flat band, u (PS,361) bf16 =
    U.flat band; out o (PS,) fp32 = rowdot(u,v)/rowsum(v)."""
    import concourse.bacc as bacc
    import concourse.tile as tile
    from concourse import mybir

    import bass_rust

    f32, bf16 = _dt()
    act_id = bass_rust.ActivationFunctionType.Identity
    nc = bacc.Bacc("TRN2", target_bir_lowering=False)
    v = nc.dram_tensor("v", [PS, L2], bf16, kind="ExternalInput")
    u = nc.dram_tensor("u", [PS, L2], bf16, kind="ExternalInput")
    o = nc.dram_tensor("o", [128, NGRP * G], f32, kind="ExternalOutput")
    vr, ur = _grouped(v[:, :]), _grouped(u[:, :])
    with tile.TileContext(nc) as tc:
        with (
            tc.tile_pool(name="io", bufs=3) as pool,
            tc.tile_pool(name="pr", bufs=2) as prp,
            tc.tile_pool(name="st", bufs=3) as spool,
            tc.tile_pool(name="acc", bufs=1) as accp,
        ):
            oacc = accp.tile([128, NGRP, G], f32)
            for g in range(NGRP):
                tv = pool.tile([128, G, L2], bf16, tag="tv")
                tu = pool.tile([128, G, L2], bf16, tag="tu")
                nc.sync.dma_start(
                    out=tv[:, :, :].rearrange("k i j -> k (i j)"), in_=vr[g]
                )
                nc.sync.dma_start(
                    out=tu[:, :, :].rearrange("k i j -> k (i j)"), in_=ur[g]
                )
                s3 = spool.tile([128, G], f32, tag="s3")
                scr = prp.tile([128, G, L2], bf16, tag="scr")
                for i in range(G):
                    # Activation: row-sums of v via accumulate output
                    nc.scalar.activation(
                        out=scr[:, i, :], in_=tv[:, i, :], func=act_id,
                        accum_out=s3[:, i : i + 1],
                    )
                prod = prp.tile([128, G, L2], bf16)
                nc.vector.tensor_mul(out=prod, in0=tu, in1=tv)
                dots = spool.tile([128, G], f32, tag="dots")
                nc.vector.tensor_reduce(
                    out=dots, in_=prod,
                    axis=mybir.AxisListType.X, op=mybir.AluOpType.add,
                )
                r3 = spool.tile([128, G], f32, tag="r3")
                nc.vector.reciprocal(out=r3, in_=s3)
                nc.vector.tensor_mul(out=oacc[:, g, :], in0=dots, in1=r3)
            nc.sync.dma_start(out=o[:, :], in_=oacc)
    nc.compile()
    return nc


def _run(key, builder, in_maps, trace=False):
    from concourse.bass_utils import run_bass_kernel_spmd

    if key not in _CACHE:
        _CACHE[key] = builder()
    res = run_bass_kernel_spmd(
        _CACHE[key], in_maps, core_ids=list(range(NCORES)), trace=trace
    )
    return res


def kernel(input, kernel):
    import os

    trace = bool(int(os.environ.get("BASSBLUR_TRACE", "0")))
    inp = np.ascontiguousarray(np.asarray(input, dtype=np.float32))
    ker = np.ascontiguousarray(np.asarray(kernel, dtype=np.float32))
    X = ker.reshape(L2, P)
    Xbf = X.astype(BF16)
    Xbff = Xbf.reshape(-1)

    times = []

    # ---- launch 1: rs1 = 1 / column sums of X -------------------------
    XTbf = np.ascontiguousarray(Xbf.T)  # (P, 361) bf16
    in1 = [
        {"xT": XTbf[m * PS : (m + 1) * PS]}
        for m in range(NCORES)
    ]
    r1 = _run("k1", _build_k1, in1, trace=trace)
    rs1 = np.concatenate(
        [r["rs1"].reshape(128, NGRP, G).transpose(1, 0, 2).ravel()
         for r in r1.results]
    )
    times.append(r1.exec_time_ns)

    # ---- launch 2: per-chunk stage-2 normalize ------------------------
    # band m covers flat [NB*m, NB*(m+1)); element x there needs
    # rs1[(NB*m + x) % P]; NB % P == PS so the roll shift is PS*m.
    in2 = []
    for m in range(NCORES):
        s1b = np.resize(np.roll(rs1, -(PS * m) % P), NB).reshape(PS, L2)
        in2.append(
            {
                "a2": Xbff[NB * m : NB * (m + 1)].reshape(PS, L2),
                "s1b": s1b.astype(BF16),
            }
        )
    r2 = _run("k2", _build_k2, in2, trace=trace)
    B2 = np.concatenate([r["b2"] for r in r2.results], axis=0)  # (P, 361) bf16
    times.append(r2.exec_time_ns)

    # ---- launch 3: final dot over B2T/U flat chunks -------------------
    B2Tf = np.ascontiguousarray(B2.T).reshape(-1)
    pad = np.pad(inp[0, 2], L // 2, mode="reflect")  # (274, 274)
    from numpy.lib.stride_tricks import sliding_window_view

    U = np.ascontiguousarray(
        sliding_window_view(pad, (256, 256)).reshape(L2, P)
    )
    Ubff = U.astype(BF16).reshape(-1)
    in3 = [
        {
            "v": B2Tf[NB * m : NB * (m + 1)].reshape(PS, L2),
            "u": Ubff[NB * m : NB * (m + 1)].reshape(PS, L2),
        }
        for m in range(NCORES)
    ]
    r3 = _run("k3", _build_k3, in3, trace=trace)
    out = np.concatenate(
        [r["o"].reshape(128, NGRP, G).transpose(1, 0, 2).ravel()
         for r in r3.results]
    )
    times.append(r3.exec_time_ns)

    if trace:
        kernel._last_times_ns = times  # stash for test harness

    return out.reshape(1, 1, 256, 256).astype(np.float32)


def hw_time_estimate_ns():
    """Per-launch HW time from the instruction cost model (TimelineSim).

    NTFF/neuron-profile capture is unavailable under this axon build, so this
    is the principled substitute: the same InstructionCostModel the Tile
    scheduler uses, over the exact BIR that runs on the cores.
    """
    from concourse.timeline_sim import TimelineSim

    out = []
    for key, builder in [("k1", _build_k1), ("k2", _build_k2), ("k3", _build_k3)]:
        if key not in _CACHE:
            _CACHE[key] = builder()
        out.append(int(TimelineSim(_CACHE[key]).simulate()))
    return out


# revision 30
# speedup vs baseline: 1.1283x; 1.0568x over previous
"""BatchBlur_SV kernel for 8 Trainium2 NeuronCores (Bass/Tile).

Reference semantics (including its reshape-scrambling "bug"):
  X = ker.reshape(361, 65536)                  # (kernel-pos ab, pixel p)
  s1 = X.sum(0);  W  = X / s1                  # stage-1 per-pixel normalize
  A2 = W.flat chunks of 361; s2 = row sums;  B2 = A2 / s2     # stage 2
  A3 = (B2.T).flat chunks of 361; s3 = row sums               # stage 3
  U  = im2col(reflect_pad(input[0,2], 9)) in (ab, p) layout   # (361, 65536)
  out[r] = sum(U.flat_chunk_r * A3[r]) / s3[r]

All arithmetic runs on-device in 3 SPMD launches over 8 cores, each core
working on a 1/8 flat band. Host only slices / rolls / transposes / casts
between launches (data movement, no math).

Perf notes vs the fp32 baseline (242us):
 - all large streamed tensors are bf16 (halves DMA bytes, doubles DVE rate);
   every reduction accumulates into fp32 tiles.
 - k1 emits 1/s1 (one small reciprocal) so k2 skips its full-size
   reciprocal pass and multiplies directly.
"""

import numpy as np
import ml_dtypes

BF16 = ml_dtypes.bfloat16

P = 65536          # pixels
L = 19
L2 = 361           # kernel positions
NCORES = 8
PS = P // NCORES   # 8192 rows per core
NB = PS * L2       # flat elements per band
G = 8              # subtiles per DMA group
NGRP = PS // (128 * G)   # 8 groups per core

_CACHE: dict = {}


def _dt():
    from concourse import mybir
    return mybir.dt.float32, mybir.dt.bfloat16


def _grouped(ap):
    # (PS, L2) -> [g][k][(i j)] with row = g*1024 + k*G + i: each partition
    # holds G consecutive rows, so src/dst DMA patterns are contiguous 2D.
    return ap.rearrange("(g k i) j -> g k (i j)", g=NGRP, k=128, i=G)


def _build_k1():
    """colsum+recip kernel: in xT (PS, 361) bf16 slice of X.T
    -> out rs1 (PS,) fp32 with rs1 = 1/colsum."""
    import concourse.bacc as bacc
    import concourse.tile as tile
    from concourse import mybir

    import bass_rust

    f32, bf16 = _dt()
    act_id = bass_rust.ActivationFunctionType.Identity
    nc = bacc.Bacc("TRN2", target_bir_lowering=False)
    xT = nc.dram_tensor("xT", [PS, L2], bf16, kind="ExternalInput")
    rs1 = nc.dram_tensor("rs1", [128, NGRP * G], f32, kind="ExternalOutput")
    xr = _grouped(xT[:, :])
    with tile.TileContext(nc) as tc:
        with (
            tc.tile_pool(name="io", bufs=3) as pool,
            tc.tile_pool(name="scr", bufs=2) as scrp,
            tc.tile_pool(name="acc", bufs=1) as accp,
        ):
            acc = accp.tile([128, NGRP, G], f32)
            racc = accp.tile([128, NGRP, G], f32)
            for g in range(NGRP):
                xt = pool.tile([128, G, L2], bf16)
                nc.sync.dma_start(
                    out=xt[:, :, :].rearrange("k i j -> k (i j)"), in_=xr[g]
                )
                # per-group engine split: DVE sums rows 0..4, Act rows 5..7
                nc.vector.tensor_reduce(
                    out=acc[:, g, 0:5], in_=xt[:, 0:5, :],
                    axis=mybir.AxisListType.X, op=mybir.AluOpType.add,
                )
                scr = scrp.tile([128, 3, L2], bf16)
                for i in range(5, G):
                    nc.scalar.activation(
                        out=scr[:, i - 5, :], in_=xt[:, i, :], func=act_id,
                        accum_out=acc[:, g, i : i + 1],
                    )
            nc.vector.reciprocal(
                out=racc[:, :, :].rearrange("k g i -> k (g i)"),
                in_=acc[:, :, :].rearrange("k g i -> k (g i)"),
            )
            nc.sync.dma_start(out=rs1[:, :], in_=racc)
    nc.compile()
    return nc


def _build_k2():
    """stage-2 kernel: in a2 (PS,361) bf16 = X.flat band, rs1b (PS,361) bf16 =
    matching per-element 1/s1; out b2 (PS,361) bf16 normalized chunks."""
    import concourse.bacc as bacc
    import concourse.tile as tile
    from concourse import mybir

    import bass_rust

    f32, bf16 = _dt()
    act_id = bass_rust.ActivationFunctionType.Identity
    nc = bacc.Bacc("TRN2", target_bir_lowering=False)
    a2 = nc.dram_tensor("a2", [PS, L2], bf16, kind="ExternalInput")
    s1b = nc.dram_tensor("s1b", [PS, L2], bf16, kind="ExternalInput")
    b2 = nc.dram_tensor("b2", [PS, L2], bf16, kind="ExternalOutput")
    a2r, s1r, b2r = _grouped(a2[:, :]), _grouped(s1b[:, :]), _grouped(b2[:, :])
    with tile.TileContext(nc) as tc:
        with (
            tc.tile_pool(name="io", bufs=3) as pool,
            tc.tile_pool(name="w", bufs=3) as wpool,
            tc.tile_pool(name="st", bufs=3) as spool,
        ):
            for g in range(NGRP):
                ta = pool.tile([128, G, L2], bf16, tag="ta")
                ts = pool.tile([128, G, L2], bf16, tag="ts")
                nc.sync.dma_start(
                    out=ta[:, :, :].rearrange("k i j -> k (i j)"), in_=a2r[g]
                )
                nc.sync.dma_start(
                    out=ts[:, :, :].rearrange("k i j -> k (i j)"), in_=s1r[g]
                )
                tw = wpool.tile([128, G, L2], bf16)
                nc.vector.tensor_mul(out=tw, in0=ta, in1=ts)
                s2 = spool.tile([128, G], f32, tag="s2")
                scr = wpool.tile([128, G, L2], bf16, tag="scr")
                for i in range(G):
                    # Activation: row-sums of tw via accumulate output
                    nc.scalar.activation(
                        out=scr[:, i, :], in_=tw[:, i, :], func=act_id,
                        accum_out=s2[:, i : i + 1],
                    )
                r2 = spool.tile([128, G], f32, tag="r2")
                nc.vector.reciprocal(out=r2, in_=s2)
                for i in range(G):
                    nc.vector.tensor_scalar_mul(
                        out=tw[:, i, :], in0=tw[:, i, :],
                        scalar1=r2[:, i : i + 1],
                    )
                nc.sync.dma_start(
                    out=b2r[g], in_=tw[:, :, :].rearrange("k i j -> k (i j)")
                )
    nc.compile()
    return nc


def _build_k3():
    """final kernel: in v (PS,361) bf16 = B2T.flat band, u (PS,361) bf16 =
    U.flat band; out o (PS,) fp32 = rowdot(u,v)/rowsum(v)."""
    import concourse.bacc as bacc
    import concourse.tile as tile
    from concourse import mybir

    import bass_rust

    f32, bf16 = _dt()
    act_id = bass_rust.ActivationFunctionType.Identity
    nc = bacc.Bacc("TRN2", target_bir_lowering=False)
    v = nc.dram_tensor("v", [PS, L2], bf16, kind="ExternalInput")
    u = nc.dram_tensor("u", [PS, L2], bf16, kind="ExternalInput")
    o = nc.dram_tensor("o", [128, NGRP * G], f32, kind="ExternalOutput")
    vr, ur = _grouped(v[:, :]), _grouped(u[:, :])
    with tile.TileContext(nc) as tc:
        with (
            tc.tile_pool(name="io", bufs=3) as pool,
            tc.tile_pool(name="pr", bufs=2) as prp,
            tc.tile_pool(name="st", bufs=3) as spool,
            tc.tile_pool(name="acc", bufs=1) as accp,
        ):
            oacc = accp.tile([128, NGRP, G], f32)
            for g in range(NGRP):
                tv = pool.tile([128, G, L2], bf16, tag="tv")
                tu = pool.tile([128, G, L2], bf16, tag="tu")
                nc.sync.dma_start(
                    out=tv[:, :, :].rearrange("k i j -> k (i j)"), in_=vr[g]
                )
                nc.sync.dma_start(
                    out=tu[:, :, :].rearrange("k i j -> k (i j)"), in_=ur[g]
                )
                s3 = spool.tile([128, G], f32, tag="s3")
                scr = prp.tile([128, G, L2], bf16, tag="scr")
                for i in range(G):
                    # Activation: row-sums of v via accumulate output
                    nc.scalar.activation(
                        out=scr[:, i, :], in_=tv[:, i, :], func=act_id,
                        accum_out=s3[:, i : i + 1],
                    )
                prod = prp.tile([128, G, L2], bf16)
                nc.vector.tensor_mul(out=prod, in0=tu, in1=tv)
                dots = spool.tile([128, G], f32, tag="dots")
                nc.vector.tensor_reduce(
                    out=dots, in_=prod,
                    axis=mybir.AxisListType.X, op=mybir.AluOpType.add,
                )
                r3 = spool.tile([128, G], f32, tag="r3")
                nc.vector.reciprocal(out=r3, in_=s3)
                nc.vector.tensor_mul(out=oacc[:, g, :], in0=dots, in1=r3)
            nc.sync.dma_start(out=o[:, :], in_=oacc)
    nc.compile()
    return nc


def _run(key, builder, in_maps, trace=False):
    from concourse.bass_utils import run_bass_kernel_spmd

    if key not in _CACHE:
        _CACHE[key] = builder()
    res = run_bass_kernel_spmd(
        _CACHE[key], in_maps, core_ids=list(range(NCORES)), trace=trace
    )
    return res


def kernel(input, kernel):
    import os

    trace = bool(int(os.environ.get("BASSBLUR_TRACE", "0")))
    inp = np.ascontiguousarray(np.asarray(input, dtype=np.float32))
    ker = np.ascontiguousarray(np.asarray(kernel, dtype=np.float32))
    X = ker.reshape(L2, P)
    Xbf = X.astype(BF16)
    Xbff = Xbf.reshape(-1)

    times = []

    # ---- launch 1: rs1 = 1 / column sums of X -------------------------
    XTbf = np.ascontiguousarray(Xbf.T)  # (P, 361) bf16
    in1 = [
        {"xT": XTbf[m * PS : (m + 1) * PS]}
        for m in range(NCORES)
    ]
    r1 = _run("k1", _build_k1, in1, trace=trace)
    rs1 = np.concatenate(
        [r["rs1"].reshape(128, NGRP, G).transpose(1, 0, 2).ravel()
         for r in r1.results]
    )
    times.append(r1.exec_time_ns)

    # ---- launch 2: per-chunk stage-2 normalize ------------------------
    # band m covers flat [NB*m, NB*(m+1)); element x there needs
    # rs1[(NB*m + x) % P]; NB % P == PS so the roll shift is PS*m.
    in2 = []
    for m in range(NCORES):
        s1b = np.resize(np.roll(rs1, -(PS * m) % P), NB).reshape(PS, L2)
        in2.append(
            {
                "a2": Xbff[NB * m : NB * (m + 1)].reshape(PS, L2),
                "s1b": s1b.astype(BF16),
            }
        )
    r2 = _run("k2", _build_k2, in2, trace=trace)
    B2 = np.concatenate([r["b2"] for r in r2.results], axis=0)  # (P, 361) bf16
    times.append(r2.exec_time_ns)

    # ---- launch 3: final dot over B2T/U flat chunks -------------------
    B2Tf = np.ascontiguousarray(B2.T).reshape(-1)
    pad = np.pad(inp[0, 2], L // 2, mode="reflect")  # (274, 274)
    from numpy.lib.stride_tricks import sliding_window_view

    U = np.ascontiguousarray(
        sliding_window_view(pad, (256, 256)).reshape(L2, P)
    )
    Ubff = U.astype(BF16).reshape(-1)
    in3 = [
        {
            "v": B2Tf[NB * m : NB * (m + 1)].reshape(PS, L2),
            "u": Ubff[NB * m : NB * (m + 1)].reshape(PS, L2),
        }
        for m in range(NCORES)
    ]
    r3 = _run("k3", _build_k3, in3, trace=trace)
    out = np.concatenate(
        [r["o"].reshape(128, NGRP, G).transpose(1, 0, 2).ravel()
         for r in r3.results]
    )
    times.append(r3.exec_time_ns)

    if trace:
        kernel._last_times_ns = times  # stash for test harness

    return out.reshape(1, 1, 256, 256).astype(np.float32)


def hw_time_estimate_ns():
    """Per-launch HW time from the instruction cost model (TimelineSim).

    NTFF/neuron-profile capture is unavailable under this axon build, so this
    is the principled substitute: the same InstructionCostModel the Tile
    scheduler uses, over the exact BIR that runs on the cores.
    """
    from concourse.timeline_sim import TimelineSim

    out = []
    for key, builder in [("k1", _build_k1), ("k2", _build_k2), ("k3", _build_k3)]:
        if key not in _CACHE:
            _CACHE[key] = builder()
        out.append(int(TimelineSim(_CACHE[key]).simulate()))
    return out
